# revision 6
# baseline (speedup 1.0000x reference)
"""Trainium2 Bass kernel for attention energies + softmax.

Computes: energies = encoder_outputs[8192,4096] @ hidden[4096] ; softmax -> [1,1,8192]

Sharding: encoder_outputs split along seq_len across 8 NeuronCores
(1024 rows each). Each core streams its 16 MiB shard from HBM and
computes local energies with a fused DVE multiply+accumulate
(scalar_tensor_tensor). Cross-core coupling is only the softmax
normalizer: each core computes local (-max, sum_exp) stats, exchanges
the 8-byte pair directly SBUF->SBUF with all peers via
remote_dma_broadcast (8 one-live-slot calls, XOR-relative addressing,
so receiver column j holds the stats of core (self^j) - the combine is
permutation-invariant), rescales its local numerators
exp(e - m_local) by alpha = exp(m_local - M)/S, and writes its
1024-row output shard. The host concatenates the shards.

Structure notes (from NTFF analysis on trn2):
- eo stream is HBM-bound (~330-360 GB/s/core with all 8 cores
  streaming, ~52 us). DMA order is half-major (h0 of tiles 0..7, then
  h1 of tiles 0..7) so the fp32 PE broadcast of hidden (512-col chunks,
  ~3.3 us each) stays ahead of DVE consumption; the final tile half is
  split into two 512 KiB quarters so the last multiply is short.
- A dependency-free warmup AllGather (stats_init payload) is REQUIRED:
  a NEFF with no collective gets no aligned multi-core launch and the
  cores start with multi-ms skew (measured 4.5 ms), which the gate wait
  then eats. It also keeps ncfw warm. Its result is never read.
- The real stats exchange bypasses ncfw (28 us warm in the baseline)
  with direct remote DMA: descriptor preps are generated early on the
  idle Q7, one trigger_dma fires all 8 after the stats replication, and
  arrivals bump a remote semaphore (+2 per sender, wait >= 16).
  trigger_dma carries signals_writable=[stats_all, stats_sb]: the WAW
  edge on stats_sb orders the doorbell after the stats write (the
  deferred-read wiring does NOT do this for remote preps - v2 sent
  stale data), and the stats_all edge orders the gate copy after the
  trigger. The arrival wait itself is spliced into the Vector stream
  after Tile scheduling (the single-core scheduling sim cannot model
  remote increments and would report a false deadlock).
- The local numerators are pre-transposed to the output layout while
  the exchange is in flight, so the post-arrival path is a handful of
  [1,8] ops, an alpha broadcast, scale, and the 4 KiB store.
"""

from contextlib import ExitStack

import numpy as np

import concourse.bacc as bacc
import concourse.tile as tile
from concourse import masks, mybir
from concourse.bass_utils import run_bass_kernel_spmd

P = 128          # SBUF partitions
H = 4096         # hidden dim
S = 8192         # full seq len
NCORES = 8
SL = S // NCORES  # 1024 rows per core
T = SL // P       # 8 seq tiles per core
HH = H // 2      # half tile cols
HQ = H // 4      # quarter tile cols
MM_N = 512       # fp32 matmul moving-operand max

F32 = mybir.dt.float32
AX = mybir.AxisListType
OP = mybir.AluOpType
ACT = mybir.ActivationFunctionType


def build_kernel():
    nc = bacc.Bacc(
        "TRN2",
        target_bir_lowering=False,
        debug=False,
        num_devices=NCORES,
        num_swdge_queues=2,
    )
    hidden_d = nc.dram_tensor("hidden", [1, H], F32, kind="ExternalInput").ap()
    eo_d = nc.dram_tensor("eo", [SL, H], F32, kind="ExternalInput").ap()
    out_d = nc.dram_tensor("out", [T, P], F32, kind="ExternalOutput").ap()

    with tile.TileContext(nc) as tc, ExitStack() as ctx:
        singles = ctx.enter_context(tc.tile_pool(name="singles", bufs=1))
        psum = ctx.enter_context(tc.tile_pool(name="psum", bufs=1, space="PSUM"))
        psum8 = ctx.enter_context(tc.tile_pool(name="psum8", bufs=2, space="PSUM"))
        psum_prod = ctx.enter_context(
            tc.tile_pool(name="psum_prod", bufs=1, space="PSUM")
        )
        dram = ctx.enter_context(tc.tile_pool(name="dram", bufs=1, space="DRAM"))

        # ---- constants ----
        ident = singles.tile([P, P], F32)
        masks.make_identity(nc, ident[:])
        ones_col = singles.tile([P, 1], F32)
        nc.vector.memset(ones_col[:], 1.0)
        ones_row = singles.tile([1, P], F32)
        nc.vector.memset(ones_row[:], 1.0)
        stats_init = singles.tile([1, 8], F32)
        nc.vector.memset(stats_init[:], 0.0)

        # ---- warmup collective: aligns the 8-core launch (a NEFF without
        # any collective starts cores with multi-ms skew) and wakes ncfw.
        # Payload is stats_init only - NO data deps, fires immediately. ----
        wu_in = dram.tile([1, 8], F32)
        wu_out = dram.tile([NCORES, 8], F32)
        nc.sync.dma_start(out=wu_in[:], in_=stats_init[:])
        nc.gpsimd.collective_compute(
            "AllGather",
            OP.bypass,
            replica_groups=[list(range(NCORES))],
            ins=[wu_in[:].opt()],
            outs=[wu_out[:].opt()],
        )

        # ---- hidden: 16 KiB DMA + PE fp32 broadcast to all 128 partitions
        # (512-col chunks; chunk k lands ~13.5+3.3k us, ahead of the
        # half-major DVE consumption below). ----
        h_row = singles.tile([1, H], F32)
        nc.sync.dma_start(out=h_row[:], in_=hidden_d)
        h_sb = singles.tile([P, H], F32)
        for j in range(0, H, MM_N):
            hb_ps = psum8.tile([P, MM_N], F32, tag="hb")
            nc.tensor.matmul(hb_ps[:], ones_row[:], h_row[:, j : j + MM_N])
            nc.scalar.copy(h_sb[:, j : j + MM_N], hb_ps[:])

        # ---- stats exchange buffers + early descriptor preps ----
        stats_sb = singles.tile([P, 2], F32)   # (nm, s) replicated to 128 parts
        stats_all = singles.tile([P, 2 * NCORES], F32)
        sem_rem = nc.alloc_semaphore("stats_arrival")
        sem_loc = nc.alloc_semaphore("stats_sent")
        for j in range(NCORES):
            rdests: list = [None] * 8
            rdests[j] = (0, j)
            nc.gpsimd.remote_dma_broadcast(
                out_ap=stats_all[:, 2 * j : 2 * j + 2],
                in_ap=stats_sb[:, 0:2],
                remote_sem=sem_rem,
                local_sem=sem_loc,
                rdests=rdests,
                queue_num=1,
            )

        # ---- local energies: e[p, t] = dot(eo[t*128+p, :], hidden) ----
        # Half-major order: h0 of tiles 0..7 (tile 0 as two quarter-STTs so
        # the first multiply needs only h chunks 0-1), then h1 of tiles 0..7
        # (tile 7 as two 512 KiB quarter-DMAs + quarter-STTs for a short
        # final multiply).
        eo_t = eo_d.rearrange("(t p) h -> t p h", p=P)
        e_part = singles.tile([P, T, 3], F32)
        nc.vector.memset(e_part[:], 0.0)
        xs = [
            singles.tile([P, H], F32, name=f"x{t}", tag=f"x{t}") for t in range(T)
        ]

        def stt(t, lo, hi, slot):
            prod = psum_prod.tile([P, HH], F32, tag="prod")
            nc.vector.scalar_tensor_tensor(
                out=prod[:, 0 : hi - lo],
                in0=xs[t][:, lo:hi],
                scalar=1.0,
                in1=h_sb[:, lo:hi],
                op0=OP.mult,
                op1=OP.mult,
                accum_out=e_part[:, t, slot : slot + 1],
            )

        # first half (cols 0:2048)
        nc.sync.dma_start(out=xs[0][:, 0:HH], in_=eo_t[0, :, 0:HH])
        stt(0, 0, HQ, 0)      # needs h chunks 0-1 only
        stt(0, HQ, HH, 1)
        for t in range(1, T):
            nc.sync.dma_start(out=xs[t][:, 0:HH], in_=eo_t[t, :, 0:HH])
            stt(t, 0, HH, 0)
        # second half (cols 2048:4096)
        for t in range(T - 1):
            nc.sync.dma_start(out=xs[t][:, HH:H], in_=eo_t[t, :, HH:H])
            stt(t, HH, H, 2 if t == 0 else 1)
        t = T - 1
        for q in (2, 3):
            lo, hi = q * HQ, (q + 1) * HQ
            nc.sync.dma_start(out=xs[t][:, lo:hi], in_=eo_t[t, :, lo:hi])
            stt(t, lo, hi, q - 1)

        e_sb = singles.tile([P, T], F32)
        nc.vector.tensor_reduce(
            out=e_sb[:], in_=e_part[:], axis=AX.X, op=OP.add
        )

        # ---- local stats: nm = -max(e_local), s = sum(exp(e_local - max)) ----
        stats_pair = singles.tile([1, 2], F32)  # [nm, s]
        m1 = singles.tile([P, 1], F32)
        nc.vector.tensor_reduce(out=m1[:], in_=e_sb[:], axis=AX.X, op=OP.max)
        m1t_ps = psum.tile([1, P], F32, tag="small")
        nc.tensor.transpose(m1t_ps[:], m1[:], ident[:])
        nc.vector.tensor_reduce(
            out=stats_pair[:, 0:1], in_=m1t_ps[:], axis=AX.X, op=OP.max, negate=True
        )
        nmb_ps = psum.tile([P, 1], F32, tag="small")
        nc.tensor.matmul(nmb_ps[:], ones_row[:], stats_pair[:, 0:1])
        nmb = singles.tile([P, 1], F32)
        nc.scalar.copy(nmb[:], nmb_ps[:])
        expl = singles.tile([P, T], F32)
        srow = singles.tile([P, 1], F32)
        nc.scalar.activation(
            expl[:], e_sb[:], ACT.Exp, bias=nmb[:], scale=1.0, accum_out=srow[:]
        )
        s_ps = psum.tile([1, 1], F32, tag="small")
        nc.tensor.matmul(s_ps[:], srow[:], ones_col[:])
        nc.vector.tensor_copy(stats_pair[:, 1:2], s_ps[:])
        # replicate (nm, s) to 128 partitions for the exchange payload
        stats_bc_ps = psum.tile([P, 2], F32, tag="small")
        nc.tensor.matmul(stats_bc_ps[:], ones_row[:], stats_pair[:, 0:2])
        nc.scalar.copy(stats_sb[:, 0:2], stats_bc_ps[:])

        # ---- fire the exchange; pre-transpose numerators while in flight.
        # signals_writable: stats_sb WAW-orders the doorbell after the stats
        # replication; stats_all RAW-orders the gate copy after the trigger.
        nc.gpsimd.trigger_dma(
            count=None,
            queue_num=1,
            signals_writable=[stats_all[:], stats_sb[:]],
        )
        expl_t_ps = psum.tile([T, P], F32, tag="small")
        nc.tensor.transpose(expl_t_ps[:], expl[:], ident[:])
        expl_t_sb = singles.tile([T, P], F32)
        nc.scalar.copy(expl_t_sb[:], expl_t_ps[:])

        # ---- gate: wait for all 16 remote-sem increments (spliced below),
        # then combine the 8 stat pairs ----
        st_local = singles.tile([1, NCORES, 2], F32)
        gate = nc.vector.tensor_copy(
            st_local[:], stats_all[0:1, :].rearrange("a (j k) -> a j k", k=2)
        )
        negM = singles.tile([1, 1], F32)
        nc.vector.tensor_reduce(
            out=negM[:], in_=st_local[:, :, 0], axis=AX.X, op=OP.min
        )
        w = singles.tile([1, NCORES], F32)
        nc.scalar.activation(
            w[:], st_local[:, :, 0], ACT.Exp, bias=negM[:], scale=-1.0
        )
        w2 = singles.tile([1, NCORES], F32)
        nc.vector.tensor_tensor(
            out=w2[:], in0=w[:], in1=st_local[:, :, 1], op=OP.mult
        )
        S_sc = singles.tile([1, 1], F32)
        nc.vector.tensor_reduce(out=S_sc[:], in_=w2[:], axis=AX.X, op=OP.add)
        rinv = singles.tile([1, 1], F32)
        nc.vector.reciprocal(rinv[:], S_sc[:])

        # ---- finalize: out = expl_t * alpha, alpha = exp(m_local - M) / S ----
        alpha = singles.tile([1, 1], F32)
        nc.scalar.activation(
            alpha[:], stats_pair[:, 0:1], ACT.Exp, bias=negM[:], scale=-1.0
        )
        nc.vector.tensor_tensor(
            out=alpha[:], in0=alpha[:], in1=rinv[:], op=OP.mult
        )
        bc_ps = psum.tile([T, 1], F32, tag="small")
        nc.tensor.matmul(bc_ps[:], ones_row[:, :T], alpha[:])
        alpha_b = singles.tile([T, 1], F32)
        nc.scalar.copy(alpha_b[:], bc_ps[:])
        o_t_sb = singles.tile([T, P], F32)
        nc.vector.tensor_scalar_mul(o_t_sb[:], expl_t_sb[:], alpha_b[:])
        nc.sync.dma_start(out=out_d, in_=o_t_sb[:])

    # ---- splice the HW-only arrival wait in front of the gate copy.
    # The Tile scheduling sim is single-core and cannot model remote sem
    # increments (it would flag a false deadlock), so the wait is added
    # after scheduling.
    wait_inst = nc.vector.wait_ge(sem_rem, 2 * NCORES)
    fn = nc.m.functions[0]
    gname, wname = gate.ins.name, wait_inst.ins.name
    gblk = wblk = None
    for blk in fn.blocks:
        names = [i.name for i in blk.instructions]
        if gname in names:
            gblk = blk
        if wname in names:
            wblk = blk
    assert gblk is not None and wblk is not None
    insts = list(wblk.instructions)
    wobj = next(i for i in insts if i.name == wname)
    insts.remove(wobj)
    wblk.instructions = insts
    ginsts = list(gblk.instructions)
    gi = next(k for k, i in enumerate(ginsts) if i.name == gname)
    ginsts.insert(gi, wobj)
    gblk.instructions = ginsts

    nc.compile()
    return nc


_NC = None


def _get_nc():
    global _NC
    if _NC is None:
        _NC = build_kernel()
    return _NC


def _make_in_maps(hidden: np.ndarray, encoder_outputs: np.ndarray):
    hidden = np.ascontiguousarray(np.asarray(hidden, dtype=np.float32)).reshape(1, H)
    eo = np.ascontiguousarray(np.asarray(encoder_outputs, dtype=np.float32))
    assert eo.shape == (S, H), eo.shape
    return [
        {"hidden": hidden, "eo": eo[c * SL : (c + 1) * SL]} for c in range(NCORES)
    ]


def kernel(hidden: np.ndarray, encoder_outputs: np.ndarray) -> np.ndarray:
    nc = _get_nc()
    in_maps = _make_in_maps(hidden, encoder_outputs)
    res = run_bass_kernel_spmd(nc, in_maps, core_ids=list(range(NCORES)))
    parts = [
        np.asarray(res.results[c]["out"], dtype=np.float32).reshape(SL)
        for c in range(NCORES)
    ]
    return np.concatenate(parts).reshape(1, 1, S)


if __name__ == "__main__":
    rng = np.random.default_rng(0)
    h = rng.standard_normal((1, H), dtype=np.float32)
    eo = rng.standard_normal((S, H), dtype=np.float32)
    got = kernel(hidden=h, encoder_outputs=eo)
    e = eo.astype(np.float64) @ h.reshape(-1).astype(np.float64)
    e -= e.max()
    p = np.exp(e)
    want = (p / p.sum()).reshape(1, 1, S)
    err = np.abs(got.astype(np.float64) - want)
    rel = err.max() / np.abs(want).max()
    print("max abs err:", err.max(), "rel:", rel)
